# revision 13
# baseline (speedup 1.0000x reference)
"""Adaptive-softmax loss (nn_AdaptiveLoss) on 8 TRN2 NeuronCores.

Vocab-parallel sharding: each core owns 1/8 of the head shortlist rows and
1/8 of every tail cluster's output rows (plus a replicated copy of the 4
cluster-link rows, de-duplicated with a 1/8 weight).  Weights are
pre-transposed on the host into [d_chunk(partition), vocab(free)] layout,
zero-padded to 128-wide vocab chunks, and cast to bf16: every matvec runs
with the 128x128 weight block as the stationary operand and the feature
chunk as the 1-column moving operand, so logits land vocab-on-partitions.

Device pipeline per core:
  1. partial h = P_shard @ f  ->  AllGather(256 B)  ->  full 960-dim h
  2. raw logits for the core's vocab slice accumulate into one
     [128, 101] PSUM image (col = 128-wide vocab chunk); zero-padded
     weight cols give exact logit 0 there; link logits carry a -ln8 bias
     added by a rank-1 matmul so exp de-duplicates the 8 replicas
  3. 5 wide Exp(accum_out) -> per-cluster partial exp sums per partition;
     one wide mul+reduce against a host-built counts image -> per-
     partition sum(cnt * logit); both collect into a [128, 6] stack
  4. a ones-vector matmul folds partitions -> [6, 1] =
     [S_head, S_0..S_3, T_raw]; one 32 B AllReduce over [8, 1]
  5. subtract the exact exp(0) padding counts, then
     loss = ln(S_head) + sum_i (n_i/4096) ln(S_i) - T_raw/4096
     via one tiny coef matmul  (all on device)

The target gather is folded into a counts image (multiplicity of each
vocab id among the 4096 targets) computed on the host from the int
targets, so no indirect DMA is needed and the graph is identical on all
cores (SPMD-safe).
"""

import math
import sys

import numpy as np

sys.path.insert(0, "/opt/trn_rl_repo")

import ml_dtypes

import concourse.bacc as bacc
import concourse.mybir as mybir
import concourse.tile as tile
from concourse.bass_utils import run_bass_kernel_spmd

NCORES = 8
D = 1024
V = 100000
SHORT = 10000
TAILS = [(512, 10000), (256, 20000), (128, 40000), (64, 20000)]
CLUSTER_STARTS = [10000, 20000, 40000, 80000]
NTARGETS = 4096

HEAD_PER = SHORT // NCORES            # 1250
TAIL_PER = [c // NCORES for _, c in TAILS]   # 1250, 2500, 5000, 2500

F32 = mybir.dt.float32
BF16 = mybir.dt.bfloat16
NP_BF16 = ml_dtypes.bfloat16


def _ceil128(n):
    return (n + 127) // 128 * 128


# per-cluster padded widths and psum-image column ranges
HEAD_PADDED = _ceil128(HEAD_PER)                 # 1280
HW_WID = HEAD_PADDED + 128                       # + link chunk -> 1408
TAIL_PADDED = [_ceil128(n) for n in TAIL_PER]    # 1280, 2560, 5120, 2560
COL_HEAD = 0
COL_LINK = HEAD_PADDED // 128                    # 10
COL_TAIL = []
_c = COL_LINK + 1
for t in TAIL_PADDED:
    COL_TAIL.append(_c)
    _c += t // 128
NCOLS = _c                                       # 101
# exact exp(0)=1 padding counts per AllReduce slot (x 8 cores)
PADSUB = [
    (HEAD_PADDED - HEAD_PER + 128 - len(TAILS)) * NCORES,
    (TAIL_PADDED[0] - TAIL_PER[0]) * NCORES,
    (TAIL_PADDED[1] - TAIL_PER[1]) * NCORES,
    (TAIL_PADDED[2] - TAIL_PER[2]) * NCORES,
    (TAIL_PADDED[3] - TAIL_PER[3]) * NCORES,
    0.0, 0.0, 0.0,
]


def _build_nc():
    nc = bacc.Bacc(
        "TRN2", target_bir_lowering=False, debug=False, num_devices=NCORES
    )
    rg = [list(range(NCORES))]

    feat_d = nc.dram_tensor("feat", [128, 8], BF16, kind="ExternalInput")
    proj_d = nc.dram_tensor("proj", [128, 1024], BF16, kind="ExternalInput")
    hw_d = nc.dram_tensor("hw", [128, 8 * HW_WID], BF16, kind="ExternalInput")
    w_d = []
    for i, (h, _) in enumerate(TAILS):
        p = min(h, 128)
        w_d.append(
            nc.dram_tensor(
                f"w{i}", [p, (h // p) * TAIL_PADDED[i]], BF16,
                kind="ExternalInput",
            )
        )
    cnt_d = nc.dram_tensor("cnt", [128, NCOLS], F32, kind="ExternalInput")
    lbias_d = nc.dram_tensor("lbias", [1, 128], BF16, kind="ExternalInput")
    # fin cols: 0=maskA 1=maskB 2=coef_ln 3=coef_raw 4=padsub
    fin_d = nc.dram_tensor("fin", [8, 5], F32, kind="ExternalInput")
    out_d = nc.dram_tensor("out", [1], F32, kind="ExternalOutput")

    AX = mybir.AxisListType.X
    EXP = mybir.ActivationFunctionType.Exp
    LN = mybir.ActivationFunctionType.Ln

    with tile.TileContext(nc) as tc:
        with (
            tc.tile_pool(name="wpool", bufs=1) as wpool,
            tc.tile_pool(name="small", bufs=1) as small,
            tc.tile_pool(name="pspool", bufs=1, space="PSUM") as pspool,
            tc.tile_pool(name="dram", bufs=1, space="DRAM") as dram,
        ):
            feat = small.tile([128, 8], BF16, name="feat_sb")
            nc.sync.dma_start(feat[:], feat_d[:])
            cnt = small.tile([128, NCOLS], F32, name="cnt_sb")
            nc.sync.dma_start(cnt[:], cnt_d[:])
            lbias = small.tile([1, 128], BF16, name="lbias_sb")
            nc.sync.dma_start(lbias[:], lbias_d[:])
            fin = small.tile([8, 5], F32, name="fin_sb")
            nc.sync.dma_start(fin[:], fin_d[:])
            onecol = small.tile([1, 1], BF16, name="onecol_sb")
            nc.vector.memset(onecol[:], 1.0)
            ones128 = small.tile([128, 1], F32, name="ones128_sb")
            nc.vector.memset(ones128[:], 1.0)

            # ---- projection partial matvec + AllGather of h (bf16) ----
            proj = wpool.tile([128, 1024], BF16, name="proj_sb")
            nc.sync.dma_start(proj[:], proj_d[:])
            hpart_ps = pspool.tile([1, 128], F32, name="hpart_ps")
            for c in range(8):
                nc.tensor.matmul(
                    hpart_ps[:],
                    feat[:, c : c + 1],
                    proj[:, 128 * c : 128 * (c + 1)],
                    start=(c == 0),
                    stop=(c == 7),
                )
            hpart = small.tile([1, 128], BF16, name="hpart_sb")
            nc.vector.tensor_copy(hpart[:], hpart_ps[:])
            ag_in = dram.tile([1, 128], BF16, name="ag_in")
            nc.sync.dma_start(ag_in[:], hpart[:])
            ag_out = dram.tile([8, 128], BF16, name="ag_out", addr_space="Shared")
            nc.gpsimd.collective_compute(
                "AllGather",
                mybir.AluOpType.bypass,
                replica_groups=rg,
                ins=[ag_in[:].opt()],
                outs=[ag_out[:].opt()],
            )
            hvec = small.tile([128, 8], BF16, name="hvec_sb")
            nc.sync.dma_start(hvec[:], ag_out.rearrange("c p -> p c"))

            # ---- logits image: [128, NCOLS] psum, col = 128-vocab chunk ----
            ps = pspool.tile([128, NCOLS], F32, name="ps")

            hw_sb = wpool.tile([128, 8 * HW_WID], BF16, name="hw_sb")
            for c in range(8):
                nc.sync.dma_start(
                    hw_sb[:, c * HW_WID : (c + 1) * HW_WID],
                    hw_d[:, c * HW_WID : (c + 1) * HW_WID],
                )
            tail_sb = []
            for i, (h, _) in enumerate(TAILS):
                p = min(h, 128)
                n_chunks = h // p
                wid = TAIL_PADDED[i]
                wsb = wpool.tile([p, n_chunks * wid], BF16, name=f"w{i}_sb")
                for ci in range(n_chunks):
                    nc.sync.dma_start(
                        wsb[0:p, ci * wid : (ci + 1) * wid],
                        w_d[i][:, ci * wid : (ci + 1) * wid],
                    )
                tail_sb.append(wsb)

            def emit_col(col, wsb, kpart, cols, wwid, voff, lhs, pre=0):
                """one accumulation group: logits for vocab chunk `col`"""
                if pre:
                    # rank-1 bias: adds lbias[p] to every partition of col
                    nc.tensor.matmul(
                        ps[:, col : col + 1], lbias[:], onecol[:],
                        start=True, stop=False,
                    )
                for ci, fc in enumerate(cols):
                    nc.tensor.matmul(
                        ps[:, col : col + 1],
                        wsb[0:kpart, ci * wwid + voff : ci * wwid + voff + 128],
                        lhs[0:kpart, fc : fc + 1],
                        start=(ci == 0 and not pre),
                        stop=(ci == len(cols) - 1),
                    )

            for v in range(HEAD_PADDED // 128):
                emit_col(COL_HEAD + v, hw_sb, 128, list(range(8)), HW_WID,
                         v * 128, feat)
            emit_col(COL_LINK, hw_sb, 128, list(range(8)), HW_WID,
                     HEAD_PADDED, feat, pre=1)
            tail_cols = [[0, 1, 2, 3], [4, 5], [6], [7]]
            for i, (h, _) in enumerate(TAILS):
                p = min(h, 128)
                for v in range(TAIL_PADDED[i] // 128):
                    emit_col(COL_TAIL[i] + v, tail_sb[i], p, tail_cols[i],
                             TAIL_PADDED[i], v * 128, hvec)

            # ---- wide exp + counts dot (128 lanes) ----
            stacked = small.tile([128, 6], F32, name="stacked")
            junkexp = small.tile([128, NCOLS], F32, name="junkexp")
            ranges = [
                (COL_HEAD, COL_LINK + 1),
                (COL_TAIL[0], COL_TAIL[1]),
                (COL_TAIL[1], COL_TAIL[2]),
                (COL_TAIL[2], COL_TAIL[3]),
                (COL_TAIL[3], NCOLS),
            ]
            for s, (a, b) in enumerate(ranges):
                nc.scalar.activation(
                    junkexp[:, a:b], ps[:, a:b], EXP,
                    accum_out=stacked[:, s : s + 1],
                )
            prod = small.tile([128, NCOLS], F32, name="prod")
            nc.vector.tensor_mul(prod[:], ps[:], cnt[:])
            nc.vector.reduce_sum(stacked[:, 5:6], prod[:], axis=AX)

            # ---- fold partitions -> [6,1], AllReduce over [8,1] ----
            sums_ps = pspool.tile([6, 1], F32, name="sums_ps")
            nc.tensor.matmul(sums_ps[:], stacked[:], ones128[:],
                             start=True, stop=True)
            s8 = small.tile([8, 1], F32, name="s8")
            nc.vector.memset(s8[:], 0.0)
            nc.vector.tensor_copy(s8[0:6, :], sums_ps[:])

            ar_in = dram.tile([8, 1], F32, name="ar_in")
            nc.sync.dma_start(ar_in[:], s8[:])
            ar_out = dram.tile([8, 1], F32, name="ar_out", addr_space="Shared")
            nc.gpsimd.collective_compute(
                "AllReduce",
                mybir.AluOpType.add,
                replica_groups=rg,
                ins=[ar_in[:].opt()],
                outs=[ar_out[:].opt()],
            )
            ar_sb = small.tile([8, 1], F32, name="ar_sb")
            nc.sync.dma_start(ar_sb[:], ar_out[:])

            # ---- final combine on device ----
            # ar2 = AR result minus exact exp(0) padding counts
            ar2 = small.tile([8, 1], F32, name="ar2")
            nc.vector.tensor_sub(ar2[:], ar_sb[:], fin[:, 4:5])
            # safe = ar2*maskA + maskB  (slots 5..7 -> 1.0 so Ln is defined)
            safe = small.tile([8, 1], F32, name="safe")
            nc.vector.tensor_mul(safe[:], ar2[:], fin[:, 0:1])
            nc.vector.tensor_add(safe[:], safe[:], fin[:, 1:2])
            z = small.tile([8, 1], F32, name="z_sb")
            nc.scalar.activation(z[:], safe[:], LN)
            # loss = z . coef_ln + ar2 . coef_raw
            loss_ps = pspool.tile([1, 1], F32, name="loss_ps")
            nc.tensor.matmul(loss_ps[:], z[:], fin[:, 2:3], start=True,
                             stop=False)
            nc.tensor.matmul(loss_ps[:], ar2[:], fin[:, 3:4], start=False,
                             stop=True)
            loss = small.tile([1, 1], F32, name="loss_sb")
            nc.vector.tensor_copy(loss[:], loss_ps[:])
            nc.sync.dma_start(out_d[:], loss[0, :])

    nc.compile()
    return nc


def _to_bf16_T(rows, n_chunks, p):
    """[n, n_chunks*p] row-major (j, d) -> [p, n_chunks*n] bf16 transposed."""
    n = rows.shape[0]
    return np.ascontiguousarray(
        rows.reshape(n, n_chunks, p).transpose(2, 1, 0).reshape(p, -1)
    ).astype(NP_BF16)


def _shard_inputs(feature, targets, head_w, t0p, t0w, t1p, t1w, t2p, t2w,
                  t3p, t3w):
    f = np.asarray(feature, np.float32)
    feat = np.ascontiguousarray(f.reshape(8, 128).T).astype(NP_BF16)

    proj_full = np.zeros((1024, D), np.float32)
    proj_full[0:512] = t0p
    proj_full[512:768] = t1p
    proj_full[768:896] = t2p
    proj_full[896:960] = t3p

    m = np.bincount(np.asarray(targets).astype(np.int64), minlength=V)
    m = m.astype(np.float32)
    n_i = np.array(
        [m[s : s + c].sum() for s, (_, c) in zip(CLUSTER_STARTS, TAILS)],
        np.float32,
    )
    fin = np.zeros((8, 5), np.float32)
    fin[0:5, 0] = 1.0                      # maskA
    fin[5:8, 1] = 1.0                      # maskB
    fin[0, 2] = 1.0                        # coef_ln
    fin[1:5, 2] = n_i / NTARGETS
    fin[5, 3] = -1.0 / NTARGETS            # coef_raw
    fin[:, 4] = np.array(PADSUB, np.float32)
    # the -ln8 link bias is baked into the link logits, so the counts
    # product picks up -n_i*ln8 in T_raw; compensate with the exact
    # bf16-rounded bias value
    b8 = float(np.float32(math.log(NCORES)).astype(NP_BF16))
    fin[5, 4] = -float(n_i.sum()) * b8
    lbias = np.zeros((1, 128), np.float32)
    lbias[0, 0 : len(TAILS)] = -math.log(NCORES)
    lbias = lbias.astype(NP_BF16)

    tails_w = [t0w, t1w, t2w, t3w]
    in_maps = []
    for k in range(NCORES):
        im = {"feat": feat, "fin": fin, "lbias": lbias}
        pk = proj_full[128 * k : 128 * (k + 1)]  # [128(j), 1024(d)]
        im["proj"] = _to_bf16_T(pk, 8, 128)
        rows = np.zeros((HW_WID, D), np.float32)
        rows[0:HEAD_PER] = head_w[HEAD_PER * k : HEAD_PER * (k + 1)]
        rows[HEAD_PADDED : HEAD_PADDED + len(TAILS)] = head_w[SHORT:]
        im["hw"] = _to_bf16_T(rows, 8, 128)
        for i, (h, _) in enumerate(TAILS):
            p = min(h, 128)
            nlen = TAIL_PER[i]
            rows = np.zeros((TAIL_PADDED[i], h), np.float32)
            rows[0:nlen] = tails_w[i][nlen * k : nlen * (k + 1)]
            im[f"w{i}"] = _to_bf16_T(rows, h // p, p)
        # counts image [128, NCOLS]: [p, col] = count of vocab id at
        # (col chunk, partition p) in this core's slice
        cnt2d = np.zeros((128, NCOLS), np.float32)
        seg = np.zeros(HEAD_PADDED, np.float32)
        seg[0:HEAD_PER] = m[HEAD_PER * k : HEAD_PER * (k + 1)]
        cnt2d[:, COL_HEAD : COL_LINK] = seg.reshape(-1, 128).T
        cnt2d[0 : len(TAILS), COL_LINK] = n_i / NCORES
        for i in range(len(TAILS)):
            s = CLUSTER_STARTS[i] + TAIL_PER[i] * k
            seg = np.zeros(TAIL_PADDED[i], np.float32)
            seg[0 : TAIL_PER[i]] = m[s : s + TAIL_PER[i]]
            cnt2d[:, COL_TAIL[i] : COL_TAIL[i] + TAIL_PADDED[i] // 128] = (
                seg.reshape(-1, 128).T
            )
        im["cnt"] = cnt2d
        in_maps.append(im)
    return in_maps


_NC_CACHE = None


def _get_nc():
    global _NC_CACHE
    if _NC_CACHE is None:
        _NC_CACHE = _build_nc()
    return _NC_CACHE


def kernel(**inputs):
    nc = _get_nc()
    in_maps = _shard_inputs(**inputs)
    res = run_bass_kernel_spmd(nc, in_maps, core_ids=list(range(NCORES)))
    val = np.asarray(res.results[0]["out"]).reshape(-1)[0]
    return np.asarray(val, dtype=np.float32)
